# revision 1
# baseline (speedup 1.0000x reference)
"""Trainium2 Bass kernel for nn_DC_SpatialAttention (deformable-conv spatial attention).

Sharding: pure data-parallel over batch, 2 batch items per NeuronCore x 8 cores.
Per batch item:
  - stream x [256, 16384] f32 from HBM in 16 chunks; on the fly compute
      avg row = (ones/256) @ x                      (PE, fp32r)
      mx row  = (ln(sum exp(45x-153)) + 153)/45     (ACT exp -> PE sum -> ACT ln)
  - xc (avg,mx) kept as zero-padded [2, 134*134] bf16 image in DRAM
  - D_k = dcn0*avg + dcn1*mx (PE, K=16) -> PRE-SHIFTED per-k planes in DRAM
    (plane k's pixel (y,x) at plane row 7+y-ky, col 7+x-kx) so the window
    reads are single-stride merged DMAs; windows dsh/dodd for 4B-aligned
    even/odd column access.
  - offset conv 7x7 (im2col patches via gpsimd SWDGE, K=98 matmuls),
    512-px psum sub-chunks; bilinear weights relu-fused on ACT.
  - bilinear tent delta on DVE (bf16), val = D00 + delta (f32),
    out = PE_sum_k(mask*val) via fp32r matmul with per-chunk selectors.
Emission order A0 B0 A1 C0 B1 C1 with disjoint PSUM regions so phase A/B of
item 1 overlap phase C of item 0.
BatchNorm: per-core sums -> AllReduce over 8 cores -> affine+sigmoid -> y.

PSUM map (free f32 offsets):
  conv (C): [0:113, 0:1536]   (oy 0:512 | ox 512:1024 | mask 1024:1536)
  mean (A): [0:16, 1536:2560]
  lse  (A): [32:48, 2560:3584]
  dvx  (B): [0:49 / 64:113, 3584:4096]
  out  (C): [0:16, 2560:3584]
"""

import os
import numpy as np
import ml_dtypes

import concourse.bass as bass
import concourse.bacc as bacc
import concourse.mybir as mybir
import concourse.tile as tile
from concourse.bass_utils import run_bass_kernel_spmd

F32 = mybir.dt.float32
F32R = mybir.dt.float32r
BF16 = mybir.dt.bfloat16
AF = mybir.ActivationFunctionType
OP = mybir.AluOpType

# ---------------- problem constants (hardcoded) ----------------
B, C, H, W = 16, 256, 128, 128
HW = H * W
K2 = 49
PAD = 3
BN_EPS = 1e-5
N_CORES = 8
BPC = B // N_CORES

LSE_T = 45.0
LSE_C = 153.0
LN2 = 0.6931471805599453

PW = H + 2 * PAD                # 134 padded xc width
XCP_N = PW * PW                 # 17956
DPW = H + 8                     # 136 padded D plane width
DPN = DPW * DPW                 # 18496
WIN_R = 66
WIN_C = 130
WIN_N = WIN_R * WIN_C           # 8580

NCH = 16                        # phase-A chunks (1024 px)
ACH = 1024
NCC = 8                         # phase-C chunks (1024 px per half)
CCH = 1024
CROWS = 8
NP = 113                        # used partitions: 0:49 (half0) + 64:113 (half1)

N_TOTAL = float(B * HW)


def _ap(t, off, pairs):
    return bass.AP(t, off, [list(p) for p in pairs])


def build_program(debug=False):
    nc = bacc.Bacc("TRN2", target_bir_lowering=False, debug=False,
                   num_devices=N_CORES)

    xs = nc.dram_tensor("xs", [BPC, C, HW], F32R, kind="ExternalInput")
    wc = nc.dram_tensor("wc", [98, 147], BF16, kind="ExternalInput")
    bias_d = nc.dram_tensor("bias", [128, 3], F32, kind="ExternalInput")
    sd0_d = nc.dram_tensor("sd0", [48, 16 * K2], BF16, kind="ExternalInput")
    selA_f_d = nc.dram_tensor("selA_f", [128, 16 * NCH], F32R, kind="ExternalInput")
    selA_b_d = nc.dram_tensor("selA_b", [128, 16 * NCH], BF16, kind="ExternalInput")
    selC_f_d = nc.dram_tensor("selC_f", [128, 16 * NCC], F32R, kind="ExternalInput")
    o16_d = nc.dram_tensor("o16", [16, 1], F32, kind="ExternalInput")
    gb_d = nc.dram_tensor("gb", [1, 2], F32, kind="ExternalInput")
    cst_d = nc.dram_tensor("cst", [128, 1], F32, kind="ExternalInput")
    y_d = nc.dram_tensor("y", [BPC, HW], F32, kind="ExternalOutput")

    dp_dram = nc.dram_tensor("dp_dram", [K2 * DPN], BF16)
    xcp_dram = nc.dram_tensor("xcp_dram", [2 * XCP_N], BF16)
    cc_in = nc.dram_tensor("cc_in", [4], F32)
    cc_out = nc.dram_tensor("cc_out", [4], F32, addr_space="Shared")

    PS = nc.alloc_psum_tensor("PS", [128, 4096], F32)

    with tile.TileContext(nc) as tc:
        dsh = nc.alloc_sbuf_tensor("dsh", [128, WIN_N], BF16)
        dodd = nc.alloc_sbuf_tensor("dodd", [128, WIN_N], BF16)
        dsb = nc.alloc_sbuf_tensor("dsb", [128, 8192], BF16)
        out_sb = nc.alloc_sbuf_tensor("out_sb", [16, 2 * CCH], F32)
        accs = nc.alloc_sbuf_tensor("accs", [16, 4], F32)
        bnt = nc.alloc_sbuf_tensor("bnt", [16, 16], F32)
        wsb = nc.alloc_sbuf_tensor("wsb", [98, 147], BF16)
        bsb = nc.alloc_sbuf_tensor("bsb", [128, 3], F32)
        sd0 = nc.alloc_sbuf_tensor("sd0_s", [48, 16 * K2], BF16)
        avmx = nc.alloc_sbuf_tensor("avmx", [48, ACH], BF16)
        selA_f = nc.alloc_sbuf_tensor("selA_f_s", [128, 16 * NCH], F32R)
        selA_b = nc.alloc_sbuf_tensor("selA_b_s", [128, 16 * NCH], BF16)
        selC_f = nc.alloc_sbuf_tensor("selC_f_s", [128, 16 * NCC], F32R)
        o16 = nc.alloc_sbuf_tensor("o16_s", [16, 1], F32)
        gbs = nc.alloc_sbuf_tensor("gbs", [1, 2], F32)
        cstsb = nc.alloc_sbuf_tensor("cst_s", [128, 1], F32)
        zt = nc.alloc_sbuf_tensor("zt", [128, 1024], BF16)
        tb = [nc.alloc_sbuf_tensor(f"tb{i}", [128, CCH], BF16) for i in range(10)]
        wgt = [nc.alloc_sbuf_tensor(f"wgt{i}", [128, CCH], BF16) for i in range(8)]
        nbsb = nc.alloc_sbuf_tensor("nbsb", [128, 3], F32)
        patch = [nc.alloc_sbuf_tensor(f"patch{h}", [98, 8192], BF16)
                 for h in range(2)]
        # double-buffered per-chunk tensors (raw so hole partitions stay finite)
        mkb = [nc.alloc_sbuf_tensor(f"mk{i}", [128, CCH], F32) for i in range(2)]
        vlb = [nc.alloc_sbuf_tensor(f"vl{i}", [128, CCH], F32) for i in range(2)]
        bab = [nc.alloc_sbuf_tensor(f"ba{i}", [128, CCH], F32R) for i in range(2)]

        dma = nc.sync.dma_start
        gdma = nc.gpsimd.dma_start

        dma(wsb.ap(), wc.ap())
        dma(bsb.ap(), bias_d.ap())
        dma(sd0.ap(), sd0_d.ap())
        dma(selA_f.ap(), selA_f_d.ap())
        dma(selA_b.ap(), selA_b_d.ap())
        dma(selC_f.ap(), selC_f_d.ap())
        dma(o16.ap(), o16_d.ap())
        dma(gbs.ap(), gb_d.ap())
        dma(cstsb.ap(), cst_d.ap())
        nc.vector.tensor_scalar_mul(nbsb.ap(), bsb.ap(), -1.0)

        # one-time zero inits: PSUM, xcp border, hole-partition safety, dp pad
        nc.vector.memset(_ap(PS, 0, [[4096, 128], [1, 4096]]), 0.0)
        nc.gpsimd.memset(zt.ap(), 0.0)
        for t in [dsh, dodd, dsb, avmx] + tb + wgt + mkb + vlb:
            nc.gpsimd.memset(t.ap(), 0.0)
        for t in bab:
            nc.gpsimd.memset(t.ap().bitcast(F32), 0.0)
        ztf = _ap(zt, 0, [[1024, 128], [1, 1024]])
        per = 128 * 1024
        # zero padded-xc image in DRAM (borders stay zero forever)
        nxc = 2 * XCP_N
        for i in range(nxc // per):
            dma(_ap(xcp_dram, i * per, [[1024, 128], [1, 1024]]), ztf)
        remx = nxc - (nxc // per) * per
        if remx:
            fr = remx // 1024
            offx = (nxc // per) * per
            if fr:
                dma(_ap(xcp_dram, offx, [[1024, fr], [1, 1024]]),
                    _ap(zt, 0, [[1024, fr], [1, 1024]]))
            tailx = remx - fr * 1024
            if tailx:
                dma(_ap(xcp_dram, offx + fr * 1024, [[tailx, 1], [1, tailx]]),
                    _ap(zt, 0, [[tailx, 1], [1, tailx]]))
        ndp = K2 * DPN
        nfull = ndp // per
        for i in range(nfull):
            dma(_ap(dp_dram, i * per, [[1024, 128], [1, 1024]]), ztf)
        rem = ndp - nfull * per
        if rem:
            frows = rem // 1024
            off = nfull * per
            if frows:
                dma(_ap(dp_dram, off, [[1024, frows], [1, 1024]]),
                    _ap(zt, 0, [[1024, frows], [1, 1024]]))
            tail = rem - frows * 1024
            if tail:
                dma(_ap(dp_dram, off + frows * 1024, [[tail, 1], [1, tail]]),
                    _ap(zt, 0, [[tail, 1], [1, tail]]))

        mean_ps = _ap(PS, 1536, [[4096, 16], [1, 1024]])
        lse_ps = _ap(PS, 32 * 4096 + 2560, [[4096, 16], [1, 1024]])
        out_ps = _ap(PS, 2560, [[4096, 16], [1, 1024]])
        bn_ps = _ap(PS, 0, [[4096, 1], [1, 4]])

        with (
            tc.tile_pool(name="xp", bufs=3) as xp,
            tc.tile_pool(name="ep", bufs=2) as ep,
            tc.tile_pool(name="st", bufs=1) as stp,
        ):
            def phase_A_gen(b):
                for n in range(NCH):
                    xt = xp.tile([128, 2 * ACH], F32R, tag="xt")
                    dma(_ap(xt.tensor, xt.offset,
                            [[2 * ACH, 128], [ACH, 2], [1, ACH]]),
                        _ap(xs, b * C * HW + n * ACH,
                            [[HW, 128], [128 * HW, 2], [1, ACH]]))
                    et = ep.tile([128, 2 * ACH], BF16, tag="et")
                    nc.scalar.activation(et[:, :], xt[:, :].bitcast(F32), AF.Exp,
                                         bias=cstsb.ap(), scale=LSE_T)
                    selfa = _ap(selA_f, 16 * n, [[16 * NCH, 128], [1, 16]])
                    selba = _ap(selA_b, 16 * n, [[16 * NCH, 128], [1, 16]])
                    for s in range(2):
                        for cb in range(2):
                            sl = slice(cb * ACH + s * 512, cb * ACH + (s + 1) * 512)
                            nc.tensor.matmul(
                                _ap(PS, 1536 + s * 512, [[4096, 16], [1, 512]]),
                                selfa, xt[:, sl],
                                start=(n == 0 and cb == 0),
                                stop=(n == NCH - 1 and cb == 1))
                        for cb in range(2):
                            sl = slice(cb * ACH + s * 512, cb * ACH + (s + 1) * 512)
                            nc.tensor.matmul(
                                _ap(PS, 32 * 4096 + 2560 + s * 512,
                                    [[4096, 16], [1, 512]]),
                                selba, et[:, sl],
                                start=(n == 0 and cb == 0),
                                stop=(n == NCH - 1 and cb == 1))
                    yield
                # readouts: avg rows -> avmx[0:16]; LSE ln-split chain runs on
                # partitions 32:48 (lse psum lives there) -> avmx[32:48]
                nc.scalar.copy(avmx.ap()[0:16, :], mean_ps)
                dma(_ap(xcp_dram, 3 * PW + 3, [[8 * PW, 16], [PW, 8], [1, 128]]),
                    avmx.ap()[0:16, :])
                # mx = (ln(S)+C)/T with S up to e^87: split ln via exponent
                # extraction (ScalarE Ln only valid to 2^64):
                #   ln(S) = Eraw*ln2 - 127*ln2 + ln(M), M in [1,2)
                sf = stp.tile([48, ACH], F32, tag="lnst")
                sfv = sf[32:48, :]
                nc.scalar.copy(sfv, lse_ps)
                bits = sfv.bitcast(mybir.dt.int32)
                ef_i = stp.tile([48, ACH], mybir.dt.int32, tag="efi")
                nc.vector.tensor_scalar(ef_i[32:48, :], bits, 23, None,
                                        OP.arith_shift_right)
                mf = stp.tile([48, ACH], F32, tag="mf")
                nc.vector.tensor_scalar(mf[32:48, :].bitcast(mybir.dt.int32),
                                        bits, 0x007FFFFF, 0x3F800000,
                                        OP.bitwise_and, OP.bitwise_or)
                ef = stp.tile([48, ACH], F32, tag="lnst")
                nc.vector.tensor_copy(ef[32:48, :], ef_i[32:48, :])
                lnm = stp.tile([48, ACH], F32, tag="efi")
                nc.scalar.activation(lnm[32:48, :], mf[32:48, :], AF.Ln)
                nc.scalar.activation(mf[32:48, :], lnm[32:48, :], AF.Copy,
                                     bias=(LSE_C - 127.0 * LN2) / LSE_T,
                                     scale=1.0 / LSE_T)
                nc.vector.scalar_tensor_tensor(avmx.ap()[32:48, :],
                                               ef[32:48, :], LN2 / LSE_T,
                                               mf[32:48, :], OP.mult, OP.add)
                dma(_ap(xcp_dram, XCP_N + 3 * PW + 3,
                        [[8 * PW, 16], [PW, 8], [1, 128]]),
                    avmx.ap()[32:48, :])
                yield

            def emit_patches():
                # im2col patches on the gpsimd SWDGE queue
                for h in range(2):
                    for cb in range(2):
                        for ky in range(7):
                            gdma(_ap(patch[h], (cb * 49 + 7 * ky) * 8192,
                                     [[8192, 7], [1, 8192]]),
                                 _ap(xcp_dram, cb * XCP_N + (64 * h + ky) * PW,
                                     [[1, 7], [PW, 64], [1, 128]]))

            def phase_B(b, patch_first=False, defer_windows=False):
                if patch_first:
                    emit_patches()
                # D planes, stored PRE-SHIFTED by (3-ky, 3-kx): plane k's
                # image pixel (y,x) lives at plane row 7+y-ky, col 7+x-kx,
                # so window reads have no per-k shift and merge per half.
                # Stacked lhsT (rows 0:16 avg coefs, 32:48 mx coefs, K=48)
                # -> one matmul per (n, s).
                for n in range(NCH):
                    half = n // 8
                    for s in range(2):
                        slot = 3584 if (2 * n + s) % 2 == 0 else 1536
                        dvx = _ap(PS, 64 * half * 4096 + slot,
                                  [[4096, 49], [1, 512]])
                        nc.tensor.matmul(dvx,
                                         _ap(sd0, n * K2, [[16 * K2, 48], [1, K2]]),
                                         avmx.ap()[0:48, s * 512:(s + 1) * 512],
                                         start=True, stop=True)
                        nc.vector.tensor_copy(
                            _ap(dsb, 64 * half * 8192 + (n % 8) * 1024 + s * 512,
                                [[8192, 49], [1, 512]]),
                            dvx)
                    if n % 8 == 7:
                        # this half's dsb rows complete: shifted plane writes
                        # (gpsimd SWDGE; overlap the next half's matmuls)
                        for ky in range(7):
                            gdma(_ap(dp_dram,
                                     7 * ky * DPN + (7 - ky + 64 * half) * DPW + 7,
                                     [[DPN - 1, 7], [DPW, 64], [1, 128]]),
                                 _ap(dsb, (64 * half + 7 * ky) * 8192,
                                     [[8192, 7], [128, 64], [1, 128]]))
                if not defer_windows:
                    emit_windows()
                if not patch_first and not defer_windows:
                    emit_patches()

            def emit_windows():
                # merged window reads (each half's windows include +-1 halo
                # rows from the other half, so both halves must be written);
                # dsh on sync HWDGE, dodd on gpsimd SWDGE: parallel queues
                for half in range(2):
                    dma(_ap(dsh, 64 * half * WIN_N + 1,
                            [[WIN_N, 49], [WIN_C, WIN_R], [1, 129]]),
                        _ap(dp_dram, (64 * half + 3) * DPW + 3,
                            [[DPN, 49], [DPW, WIN_R], [1, 129]]))
                    gdma(_ap(dodd, 64 * half * WIN_N,
                             [[WIN_N, 49], [WIN_C, WIN_R], [1, 130]]),
                         _ap(dp_dram, (64 * half + 3) * DPW + 3,
                             [[DPN, 49], [DPW, WIN_R], [1, 130]]))

            def phase_C(b, feed=None, mid=None):
                for n in range(NCC):
                    vv = lambda t: _ap(t, 0, [[CCH, NP], [1, CCH]])
                    wq = wgt[4 * (n % 2): 4 * (n % 2) + 4]
                    wym, wyp, wxm, wxp = (vv(w) for w in wq)
                    for q in range(2):
                        for h in range(2):
                            for g in range(3):
                                nc.tensor.matmul(
                                    _ap(PS, 64 * h * 4096 + g * 512,
                                        [[4096, 49], [1, 512]]),
                                    _ap(wsb, g * 49, [[147, 98], [1, 49]]),
                                    _ap(patch[h], n * CCH + q * 512,
                                        [[8192, 98], [1, 512]]),
                                    start=True, stop=True)
                        npv = lambda lo: _ap(PS, lo, [[4096, NP], [1, 512]])
                        qs = slice(q * 512, (q + 1) * 512)
                        wv = lambda t: _ap(t, q * 512, [[CCH, NP], [1, 512]])
                        # bilinear weights fused on ACT: relu(±(conv+bias))
                        nc.scalar.activation(wv(wq[0]), npv(0), AF.Relu,
                                             bias=nbsb.ap()[:NP, 0:1], scale=-1.0)
                        nc.scalar.activation(wv(wq[1]), npv(0), AF.Relu,
                                             bias=bsb.ap()[:NP, 0:1])
                        nc.scalar.activation(wv(wq[2]), npv(512), AF.Relu,
                                             bias=nbsb.ap()[:NP, 1:2], scale=-1.0)
                        nc.scalar.activation(wv(wq[3]), npv(512), AF.Relu,
                                             bias=bsb.ap()[:NP, 1:2])
                        nc.scalar.activation(wv(mkb[n % 2]), npv(1024), AF.Sigmoid,
                                             bias=bsb.ap()[:NP, 2:3])
                    v = nc.vector
                    r0 = CROWS * n

                    def sle(i):
                        return _ap(dsh, (r0 + 1 + i) * WIN_C + 2,
                                   [[WIN_N, NP], [WIN_C, CROWS], [1, 128]])

                    def slo(i, j):
                        return _ap(dodd, (r0 + 1 + i) * WIN_C + 1 + j,
                                   [[WIN_N, NP], [WIN_C, CROWS], [1, 128]])

                    D00 = sle(0)
                    bp = [vv(t) for t in tb]
                    v.tensor_sub(bp[0], sle(-1), D00)               # dyA
                    v.tensor_sub(bp[1], sle(1), D00)                # dyB
                    v.tensor_mul(bp[2], wym, bp[0])                 # r1
                    v.tensor_mul(bp[3], wyp, bp[1])                 # r2
                    v.tensor_add(bp[4], bp[2], bp[3])               # S
                    v.tensor_add(bp[5], D00, bp[4])                 # Wt
                    v.tensor_sub(bp[6], slo(0, -1), bp[5])          # u1
                    v.tensor_sub(bp[7], slo(0, 1), bp[5])           # v1
                    v.tensor_sub(bp[2], slo(-1, -1), slo(0, -1))    # tm
                    v.tensor_sub(bp[3], slo(1, -1), slo(0, -1))     # tp
                    v.tensor_sub(bp[5], slo(-1, 1), slo(0, 1))      # tm2
                    v.tensor_sub(bp[8], slo(1, 1), slo(0, 1))       # tp2
                    v.tensor_mul(bp[9], wym, bp[2])                 # u2
                    v.tensor_add(bp[2], bp[6], bp[9])               # u3
                    v.tensor_mul(bp[6], wyp, bp[3])                 # u4
                    v.tensor_add(bp[3], bp[2], bp[6])               # U
                    v.tensor_mul(bp[9], wym, bp[5])                 # x2
                    v.tensor_add(bp[5], bp[7], bp[9])               # x3
                    v.tensor_mul(bp[7], wyp, bp[8])                 # x4
                    v.tensor_add(bp[8], bp[5], bp[7])               # V
                    v.tensor_mul(bp[2], wxm, bp[3])                 # r3
                    v.tensor_mul(bp[5], wxp, bp[8])                 # r4
                    v.tensor_add(bp[6], bp[2], bp[5])               # s2
                    v.tensor_add(bp[9], bp[4], bp[6])               # delta
                    v.tensor_add(vv(vlb[n % 2]), D00, bp[9])        # val (f32)
                    v.tensor_mul(vv(bab[n % 2]), vv(mkb[n % 2]), vv(vlb[n % 2]))
                    scf = _ap(selC_f, 16 * n, [[16 * NCC, NP], [1, 16]])
                    for s in range(2):
                        opv = _ap(PS, 2560 + s * 512,
                                  [[4096, 16], [1, 512]])
                        nc.tensor.matmul(
                            opv, scf,
                            _ap(bab[n % 2], s * 512, [[CCH, NP], [1, 512]]),
                            start=(n == 0), stop=(n == NCC - 1))
                    if feed is not None:
                        for _ in range(4):
                            next(feed, None)
                    if n == 3 and mid is not None:
                        mid()
                ob_v = _ap(out_sb, b * CCH, [[2 * CCH, 16], [1, CCH]])
                nc.scalar.copy(ob_v, out_ps)
                dump = stp.tile([48, ACH], F32, tag="mf")
                nc.scalar.activation(dump[0:16, :], ob_v, AF.Identity,
                                     accum_out=_ap(accs, 2 * b,
                                                   [[4, 16], [1, 1]]))
                nc.scalar.activation(dump[0:16, :], ob_v, AF.Square,
                                     accum_out=_ap(accs, 2 * b + 1,
                                                   [[4, 16], [1, 1]]))

            a0 = phase_A_gen(0)
            for _ in a0:
                pass
            phase_B(0, patch_first=True)
            a1 = phase_A_gen(1)
            for _ in range(5):
                next(a1, None)

            def mid():
                # B1's matmuls + dp writes only: dsh/dodd/patch reloads must
                # wait until C0's last chunks have consumed the old windows
                for _ in a1:
                    pass
                phase_B(1, defer_windows=True)

            phase_C(0, feed=a1, mid=mid)
            emit_patches()
            emit_windows()
            phase_C(1)

            # ---------- BN ----------
            nc.tensor.matmul(bn_ps, o16.ap(), accs.ap(), start=True, stop=True)
            bnl = _ap(bnt, 0, [[16, 1], [1, 4]])
            nc.scalar.copy(bnl, bn_ps)
            dma(cc_in.ap(), bnl)
            nc.gpsimd.collective_compute(
                "AllReduce", OP.add,
                replica_groups=[list(range(N_CORES))],
                ins=[cc_in.ap()], outs=[cc_out.ap()])
            bnr = _ap(bnt, 4, [[16, 1], [1, 4]])
            dma(bnr, cc_out.ap())
            v = nc.vector
            e = lambda i: _ap(bnt, 4 + i, [[16, 1], [1, 1]])
            t = lambda i: _ap(bnt, 8 + i, [[16, 1], [1, 1]])
            v.tensor_add(t(0), e(0), e(2))                  # s1
            v.tensor_add(t(1), e(1), e(3))                  # s2
            v.tensor_scalar_mul(t(2), t(0), 1.0 / N_TOTAL)  # mean
            v.tensor_scalar_mul(t(3), t(1), 1.0 / N_TOTAL)  # E[x^2]
            v.tensor_mul(t(4), t(2), t(2))
            v.tensor_sub(t(5), t(3), t(4))                  # var
            v.tensor_scalar_add(t(5), t(5), BN_EPS)
            v.reciprocal(t(6), t(5))
            nc.scalar.sqrt(t(7), t(6))                      # rstd
            v.tensor_mul(_ap(bnt, 2, [[16, 1], [1, 1]]), t(7),
                         gbs.ap()[:, 0:1])                  # scale @ [0,2]
            v.tensor_mul(t(4), t(2), _ap(bnt, 2, [[16, 1], [1, 1]]))
            v.tensor_sub(_ap(bnt, 3, [[16, 1], [1, 1]]),
                         gbs.ap()[:, 1:2], t(4))            # bias @ [0,3]
            sb2 = _ap(bnt, 2, [[16, 1], [1, 2]])
            bc16 = _ap(bnt, 8, [[16, 16], [1, 2]])
            nc.gpsimd.partition_broadcast(bc16, sb2, channels=16)
            for b in range(BPC):
                yb = stp.tile([48, ACH], F32, tag="lnst")
                yb = yb[0:16, :]
                nc.scalar.activation(yb,
                                     _ap(out_sb, b * CCH,
                                         [[2 * CCH, 16], [1, CCH]]),
                                     AF.Sigmoid,
                                     bias=_ap(bnt, 9, [[16, 16], [1, 1]]),
                                     scale=_ap(bnt, 8, [[16, 16], [1, 1]]))
                dma(_ap(y_d, b * HW, [[1024, 8], [8192, 2], [1, 1024]]),
                    yb)

    nc.compile()
    return nc


_NC_CACHE = None


def _get_nc():
    global _NC_CACHE
    if _NC_CACHE is None:
        _NC_CACHE = build_program()
    return _NC_CACHE


def make_host_constants(w_off, b_off, w_dcn, gamma, beta):
    bf = ml_dtypes.bfloat16
    orig = np.empty(147, np.int64)
    for g in range(3):
        for kk in range(49):
            orig[g * 49 + kk] = (2 * kk, 2 * kk + 1, 98 + kk)[g]
    wof = w_off.reshape(147, 2, 7, 7)
    wcl = np.zeros((98, 147), np.float32)
    for c in range(2):
        for ky in range(7):
            for kx in range(7):
                wcl[c * 49 + 7 * ky + kx, :] = wof[orig, c, ky, kx]
    # bias over partition convention p = 64*half + k  (holes zero)
    bias_t = np.zeros((128, 3), np.float32)
    for g in range(3):
        bg = b_off[orig[g * 49:(g + 1) * 49]]
        bias_t[0:49, g] = bg
        bias_t[64:113, g] = bg
    dcn = w_dcn.reshape(2, 49).astype(np.float32)
    sd0 = np.zeros((48, 16 * K2), np.float32)
    for n in range(16):
        sd0[n, 49 * n:49 * (n + 1)] = dcn[0]
        sd0[32 + n, 49 * n:49 * (n + 1)] = dcn[1]
    # phase-A row-spread selectors [128, 16*NCH]
    selA_f = np.zeros((128, 16 * NCH), np.float32)
    selA_b = np.zeros((128, 16 * NCH), np.float32)
    for n in range(NCH):
        selA_f[:, 16 * n + n] = 1.0 / C
        selA_b[:, 16 * n + n] = 1.0
    # phase-C sum-over-k selectors [128, 16*NCC]
    selC = np.zeros((128, 16 * NCC), np.float32)
    for n in range(NCC):
        selC[0:49, 16 * n + 2 * n] = 1.0
        selC[64:113, 16 * n + 2 * n + 1] = 1.0
    return {
        "wc": wcl.astype(bf),
        "bias": bias_t,
        "sd0": sd0.astype(bf),
        "selA_f": selA_f,
        "selA_b": selA_b.astype(bf),
        "selC_f": selC,
        "o16": np.ones((16, 1), np.float32),
        "gb": np.array([[float(np.reshape(gamma, -1)[0]),
                         float(np.reshape(beta, -1)[0])]], np.float32),
        "cst": np.full((128, 1), -LSE_C, np.float32),
    }


def make_in_maps(x, w_off, b_off, w_dcn, gamma, beta):
    consts = make_host_constants(w_off, b_off, w_dcn, gamma, beta)
    in_maps = []
    for i in range(N_CORES):
        m = dict(consts)
        m["xs"] = np.ascontiguousarray(
            x[i * BPC:(i + 1) * BPC].reshape(BPC, C, HW).astype(np.float32))
        in_maps.append(m)
    return in_maps


def kernel(x, w_off, b_off, w_dcn, gamma, beta):
    x = np.asarray(x, np.float32)
    nc = _get_nc()
    in_maps = make_in_maps(x, np.asarray(w_off, np.float32),
                           np.asarray(b_off, np.float32),
                           np.asarray(w_dcn, np.float32),
                           np.asarray(gamma, np.float32),
                           np.asarray(beta, np.float32))
    trace = bool(int(os.environ.get("KERNEL_TRACE", "0")))
    res = run_bass_kernel_spmd(nc, in_maps, core_ids=list(range(N_CORES)),
                               trace=trace)
    ys = [np.asarray(res.results[i]["y"], np.float32).reshape(BPC, HW)
          for i in range(N_CORES)]
    out = np.stack(ys).reshape(B, 1, H, W)
    kernel.last_exec_time_ns = res.exec_time_ns
    return out

